# revision 14
# baseline (speedup 1.0000x reference)
"""Trainium2 Bass kernel for nn_Attention_87729001988509.

Computes, per batch element b:
    xp   = pad(x[b], F 800->960)            (implicit: padded rows never touched)
    q/k/v = gelu(xp @ W{q,k,v})             [S=2048, E=200]
    out[b] = softmax(q @ k.T / sqrt(960)) @ v

Sharding: data-parallel over batch. 16 batch elements -> 2 per core x 8 cores,
weights replicated. Pure SPMD, no collectives.

Per-core kernel strategy (all fp32 storage, float32r matmuls):
  - x tiles loaded naturally [128s, 800f], PE-transposed to xT [f, s].
  - QT, KT computed directly in [e, s] layout (W stationary, xT moving, N=512).
  - V computed in natural [s, e] layout (xT stationary, W moving), with an
    appended ones-column -> V' [s, 201].
  - Attention per 512-wide q-block: S^T[k, q] = KT^T-slices @ QT (PSUM),
    exp(S/sqrt(960)) via ACT (safe without max-subtraction: |scores| <~ 3),
    then out'^T[e', q] = V'^T @ expS^T accumulated over k. The ones column of
    V' yields the softmax denominator as row 200 of out'.
  - PE-transpose out' back to [q, e'], multiply by reciprocal denominator on
    DVE, DMA to DRAM.
"""

import math
from contextlib import ExitStack

import numpy as np

import concourse.bacc as bacc
import concourse.tile as tile
from concourse import mybir
from concourse import bass_utils
from concourse.masks import make_identity

# Problem shapes (hardcoded per contract).
B, S, F_IN, F_PAD, E = 16, 2048, 800, 960, 200
N_CORES = 8
B_PER_CORE = B // N_CORES
INV_SQRT_DK = float(1.0 / np.sqrt(np.float32(F_PAD)))  # 1/sqrt(960)

F32 = mybir.dt.float32
F32R = mybir.dt.float32r

# f-tiling of the 800 used input features: 6x128 + 32.
FT = [(i * 128, 128) for i in range(6)] + [(768, 32)]
# e-tiling of the 200 output features for [e, s]-layout tensors.
ET = [(0, 128), (128, 72)]
# e'-tiling of the 201 (= E + ones column) PV output rows.
EPT = [(0, 128), (128, 73)]

N_SCHUNK = 4  # s processed in 4 chunks of 512
SCH = S // N_SCHUNK  # 512
N_QBLK = 4  # q processed in 4 blocks of 512
QBL = S // N_QBLK  # 512
N_KT = S // 128  # 16 k-tiles per batch element


def build_kernel(tc, x_d, wq_d, wk_d, wv_d, out_d, repeat=1):
    nc = tc.nc
    ctx = ExitStack()

    singles = ctx.enter_context(tc.tile_pool(name="singles", bufs=1))
    xstage = ctx.enter_context(tc.tile_pool(name="xstage", bufs=6))
    xtp = ctx.enter_context(tc.tile_pool(name="xtp", bufs=14))
    qkt = ctx.enter_context(tc.tile_pool(name="qkt", bufs=1))
    vpool = ctx.enter_context(tc.tile_pool(name="vpool", bufs=16))
    estp = ctx.enter_context(tc.tile_pool(name="estp", bufs=32))
    epi = ctx.enter_context(tc.tile_pool(name="epi", bufs=3))
    ps_mm = ctx.enter_context(tc.tile_pool(name="ps_mm", bufs=3, space="PSUM"))
    ps_acc = ctx.enter_context(tc.tile_pool(name="ps_acc", bufs=1, space="PSUM"))
    ps_tp = ctx.enter_context(tc.tile_pool(name="ps_tp", bufs=3, space="PSUM"))

    # Identities for PE transposes (dtype must match the transposed data).
    ident = singles.tile([128, 128], F32, tag="ident")
    make_identity(nc, ident)
    identr = singles.tile([128, 128], F32R, tag="identr")
    nc.vector.tensor_copy(out=identr, in_=ident)

    # Replicated weights in SBUF, f-tiled. Wq/Wk as [f, 200] (lhsT for the
    # [e, s]-layout projections); Wv zero-padded to [f, 256] so the moving
    # dim of the V projection is >= 256 (float32r full-rate threshold).
    wq_sb, wk_sb, wv_sb = [], [], []
    for fb, (f0, fsz) in enumerate(FT):
        wq_t = singles.tile([fsz, E], F32R, tag=f"wq{fb}", name=f"wq{fb}")
        nc.sync.dma_start(out=wq_t, in_=wq_d[f0 : f0 + fsz, :])
        wq_sb.append(wq_t)
        wk_t = singles.tile([fsz, E], F32R, tag=f"wk{fb}", name=f"wk{fb}")
        nc.sync.dma_start(out=wk_t, in_=wk_d[f0 : f0 + fsz, :])
        wk_sb.append(wk_t)
        wv_t = singles.tile([fsz, 256], F32R, tag=f"wv{fb}", name=f"wv{fb}")
        nc.vector.memset(wv_t[:, E:256].bitcast(F32), 0.0)
        nc.sync.dma_start(out=wv_t[:, 0:E], in_=wv_d[f0 : f0 + fsz, :])
        wv_sb.append(wv_t)

    for b in [i % B_PER_CORE for i in range(B_PER_CORE * repeat)]:
        # -------- Projection phase: QT, KT [e, s] and V' [s, 201] --------
        qt = [
            qkt.tile([esz, S], F32R, tag=f"qt{et}", name=f"qt{et}_b{b}")
            for et, (e0, esz) in enumerate(ET)
        ]
        kt = [
            qkt.tile([esz, S], F32R, tag=f"kt{et}", name=f"kt{et}_b{b}")
            for et, (e0, esz) in enumerate(ET)
        ]
        v_tiles = []

        for sc in range(N_SCHUNK):
            # Build xT [f, 512] for this s-chunk via PE transposes. Four
            # transposes (one per s-tile) share one PSUM bank and drain with a
            # single wide copy, alternating DVE/ACT so neither gates the PE.
            xt = [
                xtp.tile([fsz, SCH], F32R, tag="xt", name=f"xt{fb}_b{b}c{sc}")
                for fb, (f0, fsz) in enumerate(FT)
            ]
            xns = []
            for stl in range(SCH // 128):
                s0 = sc * SCH + stl * 128
                xn = xstage.tile([128, F_IN], F32R, tag="xn", name=f"xn_b{b}s{s0}")
                nc.sync.dma_start(out=xn, in_=x_d[b, s0 : s0 + 128, :])
                xns.append(xn)
            for fb, (f0, fsz) in enumerate(FT):
                tpb = ps_tp.tile([fsz, SCH], F32R, tag="tp", name=f"xtpb{fb}")
                for stl in range(SCH // 128):
                    nc.tensor.transpose(
                        out=tpb[:, stl * 128 : (stl + 1) * 128],
                        in_=xns[stl][:, f0 : f0 + fsz],
                        identity=identr,
                    )
                if fb % 2 == 0:
                    nc.vector.tensor_copy(out=xt[fb], in_=tpb)
                else:
                    nc.scalar.copy(out=xt[fb], in_=tpb)

            # QT / KT projections: W stationary, xT moving (N=512).
            for dst, w_sb in ((qt, wq_sb), (kt, wk_sb)):
                for et, (e0, esz) in enumerate(ET):
                    ps = ps_mm.tile([esz, SCH], F32, tag="mm", name="ps_proj")
                    for fb in range(len(FT)):
                        nc.tensor.matmul(
                            ps,
                            lhsT=w_sb[fb][:, e0 : e0 + esz],
                            rhs=xt[fb],
                            start=(fb == 0),
                            stop=(fb == len(FT) - 1),
                        )
                    nc.scalar.activation(
                        out=dst[et][:, sc * SCH : (sc + 1) * SCH],
                        in_=ps,
                        func=mybir.ActivationFunctionType.Gelu,
                    )

            # V projection: xT slice stationary, Wv moving (N=256).
            for stl in range(SCH // 128):
                ps = ps_mm.tile([128, 256], F32, tag="mm", name="ps_v")
                for fb in range(len(FT)):
                    nc.tensor.matmul(
                        ps,
                        lhsT=xt[fb][:, stl * 128 : (stl + 1) * 128],
                        rhs=wv_sb[fb],
                        start=(fb == 0),
                        stop=(fb == len(FT) - 1),
                    )
                kti = sc * (SCH // 128) + stl
                vt = vpool.tile([128, E + 1], F32R, tag="v", name=f"v{kti}_b{b}")
                nc.scalar.activation(
                    out=vt[:, 0:E],
                    in_=ps[:, 0:E],
                    func=mybir.ActivationFunctionType.Gelu,
                )
                nc.vector.memset(vt[:, E : E + 1].bitcast(F32), 1.0)
                v_tiles.append(vt)

        # -------- Attention phase (software-pipelined q-blocks) --------
        # Emission order: scores+exp(qb) before PV+epilogue(qb-1), so the ACT
        # exp stream of block qb overlaps PE's PV matmuls of block qb-1 and PE
        # never waits on exp in steady state.
        def scores_exp(qb):
            q0 = qb * QBL
            est = []
            for kti in range(N_KT):
                ps = ps_mm.tile([128, QBL], F32, tag="mm", name="ps_score")
                for et, (e0, esz) in enumerate(ET):
                    nc.tensor.matmul(
                        ps,
                        lhsT=kt[et][:, kti * 128 : (kti + 1) * 128],
                        rhs=qt[et][:, q0 : q0 + QBL],
                        start=(et == 0),
                        stop=(et == len(ET) - 1),
                    )
                ex = estp.tile([128, QBL], F32R, tag="est", name=f"est{kti}")
                nc.scalar.activation(
                    out=ex,
                    in_=ps,
                    func=mybir.ActivationFunctionType.Exp,
                    scale=INV_SQRT_DK,
                )
                est.append(ex)
            return est

        def pv_epilogue(qb, est):
            q0 = qb * QBL
            # PV: out'^T[e', q] accumulated over k-tiles; row 200 = sumexp.
            o_ps = []
            for eh, (e0, esz) in enumerate(EPT):
                po = ps_acc.tile([esz, QBL], F32, tag=f"o{eh}", name=f"o{eh}_ps")
                for kti in range(N_KT):
                    nc.tensor.matmul(
                        po,
                        lhsT=v_tiles[kti][:, e0 : e0 + esz],
                        rhs=est[kti],
                        start=(kti == 0),
                        stop=(kti == N_KT - 1),
                    )
                o_ps.append(po)

            o1 = epi.tile([128, QBL], F32, tag="o1sb", name="o1sb")
            nc.vector.tensor_copy(out=o1, in_=o_ps[0])
            o2 = epi.tile([73, QBL], F32, tag="o2sb", name="o2sb")
            nc.vector.tensor_copy(out=o2, in_=o_ps[1])

            for qc in range(QBL // 128):
                c0 = qc * 128
                t1 = ps_tp.tile([128, 128], F32, tag="tp", name="t1")
                nc.tensor.transpose(out=t1, in_=o1[:, c0 : c0 + 128], identity=ident)
                t2 = ps_tp.tile([128, 73], F32, tag="tp", name="t2")
                nc.tensor.transpose(
                    out=t2, in_=o2[:, c0 : c0 + 128], identity=ident[0:73, 0:73]
                )
                rc = epi.tile([128, 1], F32, tag="rc", name="rc")
                nc.vector.reciprocal(out=rc, in_=t2[:, 72:73])
                fin = epi.tile([128, E], F32, tag="fin", name="fin")
                nc.vector.tensor_scalar_mul(out=fin[:, 0:128], in0=t1, scalar1=rc)
                nc.vector.tensor_scalar_mul(
                    out=fin[:, 128:E], in0=t2[:, 0:72], scalar1=rc
                )
                nc.sync.dma_start(
                    out=out_d[b, q0 + c0 : q0 + c0 + 128, :], in_=fin
                )

        prev = None
        for qb in range(N_QBLK):
            est = scores_exp(qb)
            if prev is not None:
                pv_epilogue(qb - 1, prev)
            prev = est
        pv_epilogue(N_QBLK - 1, prev)

    ctx.close()


_COMPILED = {}


def _get_compiled(repeat=1):
    if repeat in _COMPILED:
        return _COMPILED[repeat]

    nc = bacc.Bacc(
        "TRN2",
        target_bir_lowering=False,
        debug=False,
        num_devices=N_CORES,
    )
    x_d = nc.dram_tensor("x", (B_PER_CORE, S, F_IN), F32R, kind="ExternalInput").ap()
    wq_d = nc.dram_tensor("wq", (F_IN, E), F32R, kind="ExternalInput").ap()
    wk_d = nc.dram_tensor("wk", (F_IN, E), F32R, kind="ExternalInput").ap()
    wv_d = nc.dram_tensor("wv", (F_IN, E), F32R, kind="ExternalInput").ap()
    out_d = nc.dram_tensor("out", (B_PER_CORE, S, E), F32, kind="ExternalOutput").ap()

    with tile.TileContext(nc) as tc:
        build_kernel(tc, x_d, wq_d, wk_d, wv_d, out_d, repeat=repeat)

    nc.compile()
    _COMPILED[repeat] = nc
    return nc


def make_in_maps(x, Wq, Wk, Wv):
    """Split full inputs into per-core input maps (data-parallel over batch)."""
    x = np.ascontiguousarray(np.asarray(x, dtype=np.float32))
    # Only the first 800 rows of W participate (x is zero-padded 800->960).
    wq = np.ascontiguousarray(np.asarray(Wq, dtype=np.float32)[:F_IN])
    wk = np.ascontiguousarray(np.asarray(Wk, dtype=np.float32)[:F_IN])
    wv = np.ascontiguousarray(np.asarray(Wv, dtype=np.float32)[:F_IN])
    in_maps = []
    for c in range(N_CORES):
        in_maps.append(
            {
                "x": np.ascontiguousarray(x[c * B_PER_CORE : (c + 1) * B_PER_CORE]),
                "wq": wq,
                "wk": wk,
                "wv": wv,
            }
        )
    return in_maps


def run(in_maps, trace=False, **kwargs):
    nc = _get_compiled()
    return bass_utils.run_bass_kernel_spmd(
        nc, in_maps, core_ids=list(range(N_CORES)), trace=trace, **kwargs
    )


def kernel(x, Wq, Wk, Wv):
    in_maps = make_in_maps(x, Wq, Wk, Wv)
    res = run(in_maps)
    out = np.concatenate([r["out"] for r in res.results], axis=0)
    return out.astype(np.float32)


if __name__ == "__main__":
    rng = np.random.default_rng(0)
    x = rng.standard_normal((B, S, F_IN), dtype=np.float32)
    wq = rng.standard_normal((F_PAD, E), dtype=np.float32) / math.sqrt(F_PAD)
    wk = rng.standard_normal((F_PAD, E), dtype=np.float32) / math.sqrt(F_PAD)
    wv = rng.standard_normal((F_PAD, E), dtype=np.float32) / math.sqrt(F_PAD)
    out = kernel(x, wq, wk, wv)
    print("out", out.shape, out.dtype, float(np.abs(out).mean()))


# revision 31
# speedup vs baseline: 52.2927x; 52.2927x over previous
"""Trainium2 Bass kernel for nn_Attention_87729001988509.

Computes, per batch element b:
    xp   = pad(x[b], F 800->960)            (implicit: padded rows never touched)
    q/k/v = gelu(xp @ W{q,k,v})             [S=2048, E=200]
    out[b] = softmax(q @ k.T / sqrt(960)) @ v

Sharding: data-parallel over batch. 16 batch elements -> 2 per core x 8 cores,
weights replicated. Pure SPMD, no collectives.

Per-core kernel strategy (all fp32 storage, float32r matmuls):
  - x tiles loaded naturally [128s, 800f], PE-transposed to xT [f, s].
  - QT, KT computed directly in [e, s] layout (W stationary, xT moving, N=512).
  - V computed in natural [s, e] layout (xT stationary, W moving), with an
    appended ones-column -> V' [s, 201].
  - Attention per 512-wide q-block: S^T[k, q] = KT^T-slices @ QT (PSUM),
    exp(S/sqrt(960)) via ACT (safe without max-subtraction: |scores| <~ 3),
    then out'^T[e', q] = V'^T @ expS^T accumulated over k. The ones column of
    V' yields the softmax denominator as row 200 of out'.
  - PE-transpose out' back to [q, e'], multiply by reciprocal denominator on
    DVE, DMA to DRAM.
"""

import math
from contextlib import ExitStack

import numpy as np

import concourse.bacc as bacc
import concourse.tile as tile
from concourse import mybir
from concourse import bass_utils
from concourse.masks import make_identity

# Problem shapes (hardcoded per contract).
B, S, F_IN, F_PAD, E = 16, 2048, 800, 960, 200
N_CORES = 8
B_PER_CORE = B // N_CORES
INV_SQRT_DK = float(1.0 / np.sqrt(np.float32(F_PAD)))  # 1/sqrt(960)

F32 = mybir.dt.float32
F32R = mybir.dt.float32r

# f-tiling of the 800 used input features: 6x128 + 32.
FT = [(i * 128, 128) for i in range(6)] + [(768, 32)]
# e-tiling of the 200 output features for [e, s]-layout tensors.
ET = [(0, 128), (128, 72)]

N_SCHUNK = 4  # s processed in 4 chunks of 512
SCH = S // N_SCHUNK  # 512
N_QBLK = 4  # q processed in 4 blocks of 512
QBL = S // N_QBLK  # 512
N_KT = S // 128  # 16 k-tiles per batch element


def build_kernel(tc, x_d, wq_d, wk_d, wv_d, out_d, repeat=1):
    nc = tc.nc
    ctx = ExitStack()

    singles = ctx.enter_context(tc.tile_pool(name="singles", bufs=1))
    xstage = ctx.enter_context(tc.tile_pool(name="xstage", bufs=8))
    xtp = ctx.enter_context(tc.tile_pool(name="xtp", bufs=14))
    qkt = ctx.enter_context(tc.tile_pool(name="qkt", bufs=1))
    vpool = ctx.enter_context(tc.tile_pool(name="vpool", bufs=16))
    estp = ctx.enter_context(tc.tile_pool(name="estp", bufs=32))
    epi = ctx.enter_context(tc.tile_pool(name="epi", bufs=3))
    ps_mm = ctx.enter_context(tc.tile_pool(name="ps_mm", bufs=3, space="PSUM"))
    ps_acc = ctx.enter_context(tc.tile_pool(name="ps_acc", bufs=2, space="PSUM"))
    ps_tp = ctx.enter_context(tc.tile_pool(name="ps_tp", bufs=3, space="PSUM"))

    # Identities for PE transposes (dtype must match the transposed data).
    ident = singles.tile([128, 128], F32, tag="ident")
    make_identity(nc, ident)
    identr = singles.tile([128, 128], F32R, tag="identr")
    nc.vector.tensor_copy(out=identr, in_=ident)

    # Dummy gelu issued first so walrus's ACT table load (~2.7us) overlaps the
    # startup DMAs instead of stalling the first real gelu/copy.
    warm = singles.tile([1, 1], F32, tag="warm")
    nc.scalar.activation(
        out=warm, in_=ident[0:1, 0:1], func=mybir.ActivationFunctionType.Gelu
    )

    # Prefetch the first s-chunk's x tiles ahead of the weight DMAs so the PE
    # transposes (which only need x + identity) start as early as possible.
    first_xns = []
    for stl in range(SCH // 128):
        xn = xstage.tile([128, F_IN], F32R, tag="xn", name=f"xn_pre{stl}")
        nc.sync.dma_start(out=xn, in_=x_d[0, stl * 128 : (stl + 1) * 128, :])
        first_xns.append(xn)

    # Replicated weights in SBUF, f-tiled. Wq/Wk as [f, 200] (lhsT for the
    # [e, s]-layout projections); Wv zero-padded to [f, 256] so the moving
    # dim of the V projection is >= 256 (float32r full-rate threshold).
    # Wq/Wk land as one wide tile [128, 6*200] (f-tiles 0..5 stacked along the
    # free dim via a strided DMA) plus a [32, 200] remainder -- fewer, larger
    # startup DMAs. Wv keeps per-tile layout for its 256-pad. Wq first (first
    # consumer), then Wk, then Wv.
    wq_sb, wk_sb, wv_sb = [], [], []

    def load_wqk(w_d, pfx):
        big = singles.tile([128, 6, E], F32R, tag=f"{pfx}big", name=f"{pfx}big")
        nc.sync.dma_start(
            out=big, in_=w_d[0:768, :].rearrange("(t p) e -> p t e", p=128)
        )
        rem = singles.tile([32, E], F32R, tag=f"{pfx}rem", name=f"{pfx}rem")
        nc.sync.dma_start(out=rem, in_=w_d[768:800, :])
        return [big[:, fb, :] for fb in range(6)] + [rem]

    wq_sb = load_wqk(wq_d, "wq")
    wk_sb = load_wqk(wk_d, "wk")
    for fb, (f0, fsz) in enumerate(FT):
        wv_t = singles.tile([fsz, 256], F32R, tag=f"wv{fb}", name=f"wv{fb}")
        nc.vector.memset(wv_t[:, E:256].bitcast(F32), 0.0)
        nc.sync.dma_start(out=wv_t[:, 0:E], in_=wv_d[f0 : f0 + fsz, :])
        wv_sb.append(wv_t)

    for b in [i % B_PER_CORE for i in range(B_PER_CORE * repeat)]:
        # -------- Projection phase: QT, KT [e, s] and V' [s, 201] --------
        qt = [
            qkt.tile([esz, S], F32R, tag=f"qt{et}", name=f"qt{et}_b{b}")
            for et, (e0, esz) in enumerate(ET)
        ]
        kt = [
            qkt.tile([esz, S], F32R, tag=f"kt{et}", name=f"kt{et}_b{b}")
            for et, (e0, esz) in enumerate(ET)
        ]
        v_tiles = []

        for sc in range(N_SCHUNK):
            # Build xT [f, 512] for this s-chunk via PE transposes. Four
            # transposes (one per s-tile) share one PSUM bank and drain with a
            # single wide copy, alternating DVE/ACT so neither gates the PE.
            xt = [
                xtp.tile([fsz, SCH], F32R, tag="xt", name=f"xt{fb}_b{b}c{sc}")
                for fb, (f0, fsz) in enumerate(FT)
            ]
            if b == 0 and sc == 0 and first_xns is not None:
                xns, first_xns = first_xns, None
            else:
                xns = []
                for stl in range(SCH // 128):
                    s0 = sc * SCH + stl * 128
                    xn = xstage.tile([128, F_IN], F32R, tag="xn", name=f"xn_b{b}s{s0}")
                    nc.sync.dma_start(out=xn, in_=x_d[b, s0 : s0 + 128, :])
                    xns.append(xn)
            for fb, (f0, fsz) in enumerate(FT):
                tpb = ps_tp.tile([fsz, SCH], F32R, tag="tp", name=f"xtpb{fb}")
                for stl in range(SCH // 128):
                    nc.tensor.transpose(
                        out=tpb[:, stl * 128 : (stl + 1) * 128],
                        in_=xns[stl][:, f0 : f0 + fsz],
                        identity=identr,
                    )
                # Alternate DVE/ACT for the drain copies, except the kernel's
                # first chunk where ACT is still loading its function table.
                if (b == 0 and sc == 0) or fb % 2 == 0:
                    nc.vector.tensor_copy(out=xt[fb], in_=tpb)
                else:
                    nc.scalar.copy(out=xt[fb], in_=tpb)

            # KT / QT projections: W stationary, xT moving (N=512). KT first:
            # the first score matmuls of the attention phase wait on the last
            # KT gelu, so get it onto the ACT queue earlier.
            for dst, w_sb in ((kt, wk_sb), (qt, wq_sb)):
                for et, (e0, esz) in enumerate(ET):
                    ps = ps_mm.tile([esz, SCH], F32, tag="mm", name="ps_proj")
                    for fb in range(len(FT)):
                        nc.tensor.matmul(
                            ps,
                            lhsT=w_sb[fb][:, e0 : e0 + esz],
                            rhs=xt[fb],
                            start=(fb == 0),
                            stop=(fb == len(FT) - 1),
                        )
                    nc.scalar.activation(
                        out=dst[et][:, sc * SCH : (sc + 1) * SCH],
                        in_=ps,
                        func=mybir.ActivationFunctionType.Gelu,
                    )

            # V projection: xT slice stationary, Wv moving (N=256).
            for stl in range(SCH // 128):
                ps = ps_mm.tile([128, 256], F32, tag="mm", name="ps_v")
                for fb in range(len(FT)):
                    nc.tensor.matmul(
                        ps,
                        lhsT=xt[fb][:, stl * 128 : (stl + 1) * 128],
                        rhs=wv_sb[fb],
                        start=(fb == 0),
                        stop=(fb == len(FT) - 1),
                    )
                kti = sc * (SCH // 128) + stl
                vt = vpool.tile([128, 256], F32R, tag="v", name=f"v{kti}_b{b}")
                nc.scalar.activation(
                    out=vt,
                    in_=ps,
                    func=mybir.ActivationFunctionType.Gelu,
                )
                nc.vector.memset(vt[:, E : E + 1].bitcast(F32), 1.0)
                v_tiles.append(vt)

        # -------- Attention phase (software-pipelined q-blocks) --------
        # Emission order: scores+exp(qb) before PV+epilogue(qb-1), so the ACT
        # exp stream of block qb overlaps PE's PV matmuls of block qb-1 and PE
        # never waits on exp in steady state.
        def scores_exp(qb):
            q0 = qb * QBL
            est = []
            for kti in range(N_KT):
                ps = ps_mm.tile([128, QBL], F32, tag="mm", name="ps_score")
                for et, (e0, esz) in enumerate(ET):
                    nc.tensor.matmul(
                        ps,
                        lhsT=kt[et][:, kti * 128 : (kti + 1) * 128],
                        rhs=qt[et][:, q0 : q0 + QBL],
                        start=(et == 0),
                        stop=(et == len(ET) - 1),
                    )
                ex = estp.tile([128, QBL], F32R, tag="est", name=f"est{kti}")
                nc.scalar.activation(
                    out=ex,
                    in_=ps,
                    func=mybir.ActivationFunctionType.Exp,
                    scale=INV_SQRT_DK,
                )
                est.append(ex)
            return est

        def pv_epilogue(qb, est):
            q0 = qb * QBL
            # PV with expS^T stationary: out[q, e'] = sum_k expS^T[k, q-tile].T
            # @ V'[k, :]. Output lands directly in [q, e] layout; column 200 is
            # the softmax denominator (ones column of V').
            for qc in range(QBL // 128):
                c0 = qc * 128
                po = ps_acc.tile([128, 256], F32, tag="o", name="o_ps")
                for kti in range(N_KT):
                    nc.tensor.matmul(
                        po,
                        lhsT=est[kti][:, c0 : c0 + 128],
                        rhs=v_tiles[kti],
                        start=(kti == 0),
                        stop=(kti == N_KT - 1),
                    )
                rc = epi.tile([128, 1], F32, tag="rc", name="rc")
                nc.vector.reciprocal(out=rc, in_=po[:, E : E + 1])
                fin = epi.tile([128, E], F32, tag="fin", name="fin")
                nc.vector.tensor_scalar_mul(out=fin, in0=po[:, 0:E], scalar1=rc)
                nc.sync.dma_start(
                    out=out_d[b, q0 + c0 : q0 + c0 + 128, :], in_=fin
                )

        prev = None
        for qb in range(N_QBLK):
            est = scores_exp(qb)
            if prev is not None:
                pv_epilogue(qb - 1, prev)
            prev = est
        pv_epilogue(N_QBLK - 1, prev)

    ctx.close()


_COMPILED = {}


def _get_compiled(repeat=1):
    if repeat in _COMPILED:
        return _COMPILED[repeat]

    nc = bacc.Bacc(
        "TRN2",
        target_bir_lowering=False,
        debug=False,
        num_devices=N_CORES,
    )
    x_d = nc.dram_tensor("x", (B_PER_CORE, S, F_IN), F32R, kind="ExternalInput").ap()
    wq_d = nc.dram_tensor("wq", (F_IN, E), F32R, kind="ExternalInput").ap()
    wk_d = nc.dram_tensor("wk", (F_IN, E), F32R, kind="ExternalInput").ap()
    wv_d = nc.dram_tensor("wv", (F_IN, E), F32R, kind="ExternalInput").ap()
    out_d = nc.dram_tensor("out", (B_PER_CORE, S, E), F32, kind="ExternalOutput").ap()

    with tile.TileContext(nc) as tc:
        build_kernel(tc, x_d, wq_d, wk_d, wv_d, out_d, repeat=repeat)

    nc.compile()
    _COMPILED[repeat] = nc
    return nc


def make_in_maps(x, Wq, Wk, Wv):
    """Split full inputs into per-core input maps (data-parallel over batch)."""
    x = np.ascontiguousarray(np.asarray(x, dtype=np.float32))
    # Only the first 800 rows of W participate (x is zero-padded 800->960).
    wq = np.ascontiguousarray(np.asarray(Wq, dtype=np.float32)[:F_IN])
    wk = np.ascontiguousarray(np.asarray(Wk, dtype=np.float32)[:F_IN])
    wv = np.ascontiguousarray(np.asarray(Wv, dtype=np.float32)[:F_IN])
    in_maps = []
    for c in range(N_CORES):
        in_maps.append(
            {
                "x": np.ascontiguousarray(x[c * B_PER_CORE : (c + 1) * B_PER_CORE]),
                "wq": wq,
                "wk": wk,
                "wv": wv,
            }
        )
    return in_maps


def run(in_maps, trace=False, **kwargs):
    nc = _get_compiled()
    return bass_utils.run_bass_kernel_spmd(
        nc, in_maps, core_ids=list(range(N_CORES)), trace=trace, **kwargs
    )


def kernel(x, Wq, Wk, Wv):
    in_maps = make_in_maps(x, Wq, Wk, Wv)
    last_err = None
    for _ in range(3):  # retry: the axon tunnel is occasionally flaky
        try:
            res = run(in_maps)
            out = np.concatenate([np.asarray(r["out"]) for r in res.results], axis=0)
            return out.astype(np.float32)
        except Exception as e:  # noqa: BLE001
            last_err = e
    raise last_err


if __name__ == "__main__":
    rng = np.random.default_rng(0)
    x = rng.standard_normal((B, S, F_IN), dtype=np.float32)
    wq = rng.standard_normal((F_PAD, E), dtype=np.float32) / math.sqrt(F_PAD)
    wk = rng.standard_normal((F_PAD, E), dtype=np.float32) / math.sqrt(F_PAD)
    wv = rng.standard_normal((F_PAD, E), dtype=np.float32) / math.sqrt(F_PAD)
    out = kernel(x, wq, wk, wv)
    print("out", out.shape, out.dtype, float(np.abs(out).mean()))


# revision 32
# speedup vs baseline: 52.3309x; 1.0007x over previous
"""Trainium2 Bass kernel for nn_Attention_87729001988509.

Computes, per batch element b:
    xp   = pad(x[b], F 800->960)            (implicit: padded rows never touched)
    q/k/v = gelu(xp @ W{q,k,v})             [S=2048, E=200]
    out[b] = softmax(q @ k.T / sqrt(960)) @ v

Sharding: data-parallel over batch. 16 batch elements -> 2 per core x 8 cores,
weights replicated. Pure SPMD, no collectives.

Per-core kernel strategy (all fp32 storage, float32r matmuls):
  - x tiles loaded naturally [128s, 800f], PE-transposed to xT [f, s].
  - QT, KT computed directly in [e, s] layout (W stationary, xT moving, N=512).
  - V computed in natural [s, e] layout (xT stationary, W moving), with an
    appended ones-column -> V' [s, 201].
  - Attention per 512-wide q-block: S^T[k, q] = KT^T-slices @ QT (PSUM),
    exp(S/sqrt(960)) via ACT (safe without max-subtraction: |scores| <~ 3),
    then out'^T[e', q] = V'^T @ expS^T accumulated over k. The ones column of
    V' yields the softmax denominator as row 200 of out'.
  - PE-transpose out' back to [q, e'], multiply by reciprocal denominator on
    DVE, DMA to DRAM.
"""

import math
from contextlib import ExitStack

import numpy as np

import concourse.bacc as bacc
import concourse.tile as tile
from concourse import mybir
from concourse import bass_utils
from concourse.masks import make_identity

# Problem shapes (hardcoded per contract).
B, S, F_IN, F_PAD, E = 16, 2048, 800, 960, 200
N_CORES = 8
B_PER_CORE = B // N_CORES
INV_SQRT_DK = float(1.0 / np.sqrt(np.float32(F_PAD)))  # 1/sqrt(960)

F32 = mybir.dt.float32
F32R = mybir.dt.float32r

# f-tiling of the 800 used input features: 6x128 + 32.
FT = [(i * 128, 128) for i in range(6)] + [(768, 32)]
# e-tiling of the 200 output features for [e, s]-layout tensors.
ET = [(0, 128), (128, 72)]

N_SCHUNK = 4  # s processed in 4 chunks of 512
SCH = S // N_SCHUNK  # 512
N_QBLK = 4  # q processed in 4 blocks of 512
QBL = S // N_QBLK  # 512
N_KT = S // 128  # 16 k-tiles per batch element


def build_kernel(tc, x_d, wq_d, wk_d, wv_d, out_d, repeat=1):
    nc = tc.nc
    ctx = ExitStack()

    singles = ctx.enter_context(tc.tile_pool(name="singles", bufs=1))
    xstage = ctx.enter_context(tc.tile_pool(name="xstage", bufs=8))
    xtp = ctx.enter_context(tc.tile_pool(name="xtp", bufs=14))
    qkt = ctx.enter_context(tc.tile_pool(name="qkt", bufs=1))
    vpool = ctx.enter_context(tc.tile_pool(name="vpool", bufs=16))
    estp = ctx.enter_context(tc.tile_pool(name="estp", bufs=32))
    epi = ctx.enter_context(tc.tile_pool(name="epi", bufs=3))
    ps_mm = ctx.enter_context(tc.tile_pool(name="ps_mm", bufs=3, space="PSUM"))
    ps_acc = ctx.enter_context(tc.tile_pool(name="ps_acc", bufs=2, space="PSUM"))
    ps_tp = ctx.enter_context(tc.tile_pool(name="ps_tp", bufs=3, space="PSUM"))

    # Identities for PE transposes (dtype must match the transposed data).
    ident = singles.tile([128, 128], F32, tag="ident")
    make_identity(nc, ident)
    identr = singles.tile([128, 128], F32R, tag="identr")
    nc.vector.tensor_copy(out=identr, in_=ident)

    # Dummy gelu issued first so walrus's ACT table load (~2.7us) overlaps the
    # startup DMAs instead of stalling the first real gelu/copy.
    warm = singles.tile([1, 1], F32, tag="warm")
    nc.scalar.activation(
        out=warm, in_=ident[0:1, 0:1], func=mybir.ActivationFunctionType.Gelu
    )

    # Prefetch the first s-chunk's x tiles ahead of the weight DMAs so the PE
    # transposes (which only need x + identity) start as early as possible.
    first_xns = []
    for stl in range(SCH // 128):
        xn = xstage.tile([128, F_IN], F32R, tag="xn", name=f"xn_pre{stl}")
        nc.sync.dma_start(out=xn, in_=x_d[0, stl * 128 : (stl + 1) * 128, :])
        first_xns.append(xn)

    # Replicated weights in SBUF, f-tiled. Wq/Wk as [f, 200] (lhsT for the
    # [e, s]-layout projections); Wv zero-padded to [f, 256] so the moving
    # dim of the V projection is >= 256 (float32r full-rate threshold).
    # Wq/Wk land as one wide tile [128, 6*200] (f-tiles 0..5 stacked along the
    # free dim via a strided DMA) plus a [32, 200] remainder -- fewer, larger
    # startup DMAs. Wv keeps per-tile layout for its 256-pad. Wq first (first
    # consumer), then Wk, then Wv.
    wq_sb, wk_sb, wv_sb = [], [], []

    def load_wqk(w_d, pfx):
        big = singles.tile([128, 6, E], F32R, tag=f"{pfx}big", name=f"{pfx}big")
        nc.sync.dma_start(
            out=big, in_=w_d[0:768, :].rearrange("(t p) e -> p t e", p=128)
        )
        rem = singles.tile([32, E], F32R, tag=f"{pfx}rem", name=f"{pfx}rem")
        nc.sync.dma_start(out=rem, in_=w_d[768:800, :])
        return [big[:, fb, :] for fb in range(6)] + [rem]

    wq_sb = load_wqk(wq_d, "wq")
    wk_sb = load_wqk(wk_d, "wk")
    for fb, (f0, fsz) in enumerate(FT):
        wv_t = singles.tile([fsz, 256], F32R, tag=f"wv{fb}", name=f"wv{fb}")
        nc.vector.memset(wv_t[:, E:256].bitcast(F32), 0.0)
        nc.sync.dma_start(out=wv_t[:, 0:E], in_=wv_d[f0 : f0 + fsz, :])
        wv_sb.append(wv_t)

    for b in [i % B_PER_CORE for i in range(B_PER_CORE * repeat)]:
        # -------- Projection phase: QT, KT [e, s] and V' [s, 201] --------
        qt = [
            qkt.tile([esz, S], F32R, tag=f"qt{et}", name=f"qt{et}_b{b}")
            for et, (e0, esz) in enumerate(ET)
        ]
        kt = [
            qkt.tile([esz, S], F32R, tag=f"kt{et}", name=f"kt{et}_b{b}")
            for et, (e0, esz) in enumerate(ET)
        ]
        v_tiles = []

        for sc in range(N_SCHUNK):
            # Build xT [f, 512] for this s-chunk via PE transposes. Four
            # transposes (one per s-tile) share one PSUM bank and drain with a
            # single wide copy, alternating DVE/ACT so neither gates the PE.
            xt = [
                xtp.tile([fsz, SCH], F32R, tag="xt", name=f"xt{fb}_b{b}c{sc}")
                for fb, (f0, fsz) in enumerate(FT)
            ]
            if b == 0 and sc == 0 and first_xns is not None:
                xns, first_xns = first_xns, None
            else:
                xns = []
                for stl in range(SCH // 128):
                    s0 = sc * SCH + stl * 128
                    xn = xstage.tile([128, F_IN], F32R, tag="xn", name=f"xn_b{b}s{s0}")
                    nc.sync.dma_start(out=xn, in_=x_d[b, s0 : s0 + 128, :])
                    xns.append(xn)
            for fb, (f0, fsz) in enumerate(FT):
                tpb = ps_tp.tile([fsz, SCH], F32R, tag="tp", name=f"xtpb{fb}")
                for stl in range(SCH // 128):
                    nc.tensor.transpose(
                        out=tpb[:, stl * 128 : (stl + 1) * 128],
                        in_=xns[stl][:, f0 : f0 + fsz],
                        identity=identr,
                    )
                # Alternate DVE/ACT for the drain copies so neither engine
                # gates the PE transpose stream.
                if fb % 2 == 0:
                    nc.vector.tensor_copy(out=xt[fb], in_=tpb)
                else:
                    nc.scalar.copy(out=xt[fb], in_=tpb)

            # KT / QT projections: W stationary, xT moving (N=512). KT first:
            # the first score matmuls of the attention phase wait on the last
            # KT gelu, so get it onto the ACT queue earlier.
            for dst, w_sb in ((kt, wk_sb), (qt, wq_sb)):
                for et, (e0, esz) in enumerate(ET):
                    ps = ps_mm.tile([esz, SCH], F32, tag="mm", name="ps_proj")
                    for fb in range(len(FT)):
                        nc.tensor.matmul(
                            ps,
                            lhsT=w_sb[fb][:, e0 : e0 + esz],
                            rhs=xt[fb],
                            start=(fb == 0),
                            stop=(fb == len(FT) - 1),
                        )
                    nc.scalar.activation(
                        out=dst[et][:, sc * SCH : (sc + 1) * SCH],
                        in_=ps,
                        func=mybir.ActivationFunctionType.Gelu,
                    )

            # V projection: xT slice stationary, Wv moving (N=256).
            for stl in range(SCH // 128):
                ps = ps_mm.tile([128, 256], F32, tag="mm", name="ps_v")
                for fb in range(len(FT)):
                    nc.tensor.matmul(
                        ps,
                        lhsT=xt[fb][:, stl * 128 : (stl + 1) * 128],
                        rhs=wv_sb[fb],
                        start=(fb == 0),
                        stop=(fb == len(FT) - 1),
                    )
                kti = sc * (SCH // 128) + stl
                vt = vpool.tile([128, 256], F32R, tag="v", name=f"v{kti}_b{b}")
                nc.scalar.activation(
                    out=vt,
                    in_=ps,
                    func=mybir.ActivationFunctionType.Gelu,
                )
                nc.vector.memset(vt[:, E : E + 1].bitcast(F32), 1.0)
                v_tiles.append(vt)

        # -------- Attention phase (software-pipelined q-blocks) --------
        # Emission order: scores+exp(qb) before PV+epilogue(qb-1), so the ACT
        # exp stream of block qb overlaps PE's PV matmuls of block qb-1 and PE
        # never waits on exp in steady state.
        def scores_exp(qb):
            q0 = qb * QBL
            est = []
            for kti in range(N_KT):
                ps = ps_mm.tile([128, QBL], F32, tag="mm", name="ps_score")
                for et, (e0, esz) in enumerate(ET):
                    nc.tensor.matmul(
                        ps,
                        lhsT=kt[et][:, kti * 128 : (kti + 1) * 128],
                        rhs=qt[et][:, q0 : q0 + QBL],
                        start=(et == 0),
                        stop=(et == len(ET) - 1),
                    )
                ex = estp.tile([128, QBL], F32R, tag="est", name=f"est{kti}")
                nc.scalar.activation(
                    out=ex,
                    in_=ps,
                    func=mybir.ActivationFunctionType.Exp,
                    scale=INV_SQRT_DK,
                )
                est.append(ex)
            return est

        def pv_epilogue(qb, est):
            q0 = qb * QBL
            # PV with expS^T stationary: out[q, e'] = sum_k expS^T[k, q-tile].T
            # @ V'[k, :]. Output lands directly in [q, e] layout; column 200 is
            # the softmax denominator (ones column of V').
            for qc in range(QBL // 128):
                c0 = qc * 128
                po = ps_acc.tile([128, 256], F32, tag="o", name="o_ps")
                for kti in range(N_KT):
                    nc.tensor.matmul(
                        po,
                        lhsT=est[kti][:, c0 : c0 + 128],
                        rhs=v_tiles[kti],
                        start=(kti == 0),
                        stop=(kti == N_KT - 1),
                    )
                rc = epi.tile([128, 1], F32, tag="rc", name="rc")
                nc.vector.reciprocal(out=rc, in_=po[:, E : E + 1])
                fin = epi.tile([128, E], F32, tag="fin", name="fin")
                nc.vector.tensor_scalar_mul(out=fin, in0=po[:, 0:E], scalar1=rc)
                nc.sync.dma_start(
                    out=out_d[b, q0 + c0 : q0 + c0 + 128, :], in_=fin
                )

        prev = None
        for qb in range(N_QBLK):
            est = scores_exp(qb)
            if prev is not None:
                pv_epilogue(qb - 1, prev)
            prev = est
        pv_epilogue(N_QBLK - 1, prev)

    ctx.close()


_COMPILED = {}


def _get_compiled(repeat=1):
    if repeat in _COMPILED:
        return _COMPILED[repeat]

    nc = bacc.Bacc(
        "TRN2",
        target_bir_lowering=False,
        debug=False,
        num_devices=N_CORES,
    )
    x_d = nc.dram_tensor("x", (B_PER_CORE, S, F_IN), F32R, kind="ExternalInput").ap()
    wq_d = nc.dram_tensor("wq", (F_IN, E), F32R, kind="ExternalInput").ap()
    wk_d = nc.dram_tensor("wk", (F_IN, E), F32R, kind="ExternalInput").ap()
    wv_d = nc.dram_tensor("wv", (F_IN, E), F32R, kind="ExternalInput").ap()
    out_d = nc.dram_tensor("out", (B_PER_CORE, S, E), F32, kind="ExternalOutput").ap()

    with tile.TileContext(nc) as tc:
        build_kernel(tc, x_d, wq_d, wk_d, wv_d, out_d, repeat=repeat)

    nc.compile()
    _COMPILED[repeat] = nc
    return nc


def make_in_maps(x, Wq, Wk, Wv):
    """Split full inputs into per-core input maps (data-parallel over batch)."""
    x = np.ascontiguousarray(np.asarray(x, dtype=np.float32))
    # Only the first 800 rows of W participate (x is zero-padded 800->960).
    wq = np.ascontiguousarray(np.asarray(Wq, dtype=np.float32)[:F_IN])
    wk = np.ascontiguousarray(np.asarray(Wk, dtype=np.float32)[:F_IN])
    wv = np.ascontiguousarray(np.asarray(Wv, dtype=np.float32)[:F_IN])
    in_maps = []
    for c in range(N_CORES):
        in_maps.append(
            {
                "x": np.ascontiguousarray(x[c * B_PER_CORE : (c + 1) * B_PER_CORE]),
                "wq": wq,
                "wk": wk,
                "wv": wv,
            }
        )
    return in_maps


def run(in_maps, trace=False, **kwargs):
    nc = _get_compiled()
    return bass_utils.run_bass_kernel_spmd(
        nc, in_maps, core_ids=list(range(N_CORES)), trace=trace, **kwargs
    )


def kernel(x, Wq, Wk, Wv):
    in_maps = make_in_maps(x, Wq, Wk, Wv)
    last_err = None
    for _ in range(3):  # retry: the axon tunnel is occasionally flaky
        try:
            res = run(in_maps)
            out = np.concatenate([np.asarray(r["out"]) for r in res.results], axis=0)
            return out.astype(np.float32)
        except Exception as e:  # noqa: BLE001
            last_err = e
    raise last_err


if __name__ == "__main__":
    rng = np.random.default_rng(0)
    x = rng.standard_normal((B, S, F_IN), dtype=np.float32)
    wq = rng.standard_normal((F_PAD, E), dtype=np.float32) / math.sqrt(F_PAD)
    wk = rng.standard_normal((F_PAD, E), dtype=np.float32) / math.sqrt(F_PAD)
    wv = rng.standard_normal((F_PAD, E), dtype=np.float32) / math.sqrt(F_PAD)
    out = kernel(x, wq, wk, wv)
    print("out", out.shape, out.dtype, float(np.abs(out).mean()))
